# revision 38
# baseline (speedup 1.0000x reference)
"""VQ codebook quantization kernel for 8 TRN2 NeuronCores.

Data-parallel: inputs [131072, 64] sharded 16384 rows/core, codebook [512, 64]
replicated. Per 128-row tile the device computes:
  - s = x @ (2*emb)^T - B   via one PE matmul with an augmented contraction
    row (lhsT row 64 = ones, rhs row 64 = -B), where B = sum(emb^2) per code
  - argmax(s) (max8 + max_index), one-hot encodings, quantized rows (indirect
    DMA gather), and per-row sum(x^2) for the loss.
The reference computes distances in f32 at magnitude ||x||^2 ~ 64, which
quantizes them to a ~7.6e-6 grid and creates near-ties; the device ranks by
the fine-grained s instead, and the host repairs the ~0.5% of rows whose
top-2 gap is below a threshold by recomputing them with the reference's exact
f32 rounding semantics.
"""

import numpy as np

import concourse.bass as bass
import concourse.mybir as mybir
from concourse.bacc import Bacc
from concourse.tile import TileContext
from concourse.bass_utils import run_bass_kernel_spmd

N_CORES = 8
N, D, K = 131072, 64, 512
NS = N // N_CORES            # rows per core
COMMITMENT_COST = 0.25
GAP_THRESHOLD = 5e-5         # flag rows for host repair

F32 = mybir.dt.float32
F32R = mybir.dt.float32r
U32 = mybir.dt.uint32


def build_nc(ns=NS):
    nt = ns // 128           # tiles per core
    nc = Bacc()
    x_in = nc.declare_dram_parameter("x", [ns, D], F32, isOutput=False)
    e_in = nc.declare_dram_parameter("emb", [K, D], F32, isOutput=False)
    et2_in = nc.declare_dram_parameter("embT2B", [D + 1, K], F32,
                                       isOutput=False)
    enc_o = nc.declare_dram_parameter("enc", [ns, K], F32, isOutput=True)
    q_o = nc.declare_dram_parameter("q", [ns, D], F32, isOutput=True)
    idx_o = nc.declare_dram_parameter("idx", [128, 8 * nt], U32, isOutput=True)
    sm_o = nc.declare_dram_parameter("sm", [128, 8 * nt], F32, isOutput=True)

    with TileContext(nc) as tc:
        with (
            tc.tile_pool(name="persist", bufs=1) as pp,
            tc.tile_pool(name="psum_t", bufs=3, space="PSUM") as ptp,
            tc.tile_pool(name="psum_m", bufs=3, space="PSUM") as pmp,
            tc.tile_pool(name="xp", bufs=5) as xp,
            tc.tile_pool(name="xtp", bufs=3) as xtp,
            tc.tile_pool(name="sp", bufs=4) as sp,
            tc.tile_pool(name="encp", bufs=4) as encp,
            tc.tile_pool(name="qp", bufs=4) as qp,
            tc.tile_pool(name="smallp", bufs=4) as smp,
        ):
            # ---------------- persistent tiles ----------------
            ident = pp.tile([128, 128], F32)        # identity for PE transpose
            embT2B = pp.tile([D + 1, K], F32)       # rows 0-63: (2emb)^T; 64: -B
            embT2Br = pp.tile([D + 1, K], F32R)     # f32r-rounded copy
            iota512 = pp.tile([128, K], F32)        # 0..511 each row
            smax_all = pp.tile([128, 8 * nt], F32)  # max8 outputs per tile
            idx_all = pp.tile([128, 8 * nt], U32)   # max_index outputs
            xt_bufs = [pp.tile([D + 1, 128], F32R, tag=f"xtb{i}",
                               name=f"xtb{i}")
                       for i in range(4)]

            # ---------------- setup ----------------
            iota_row = pp.tile([128, 128], F32)
            iota_col = pp.tile([128, 1], F32)
            nc.gpsimd.iota(iota_row[:, :], pattern=[[1, 128]], base=0,
                           channel_multiplier=0,
                           allow_small_or_imprecise_dtypes=True)
            nc.gpsimd.iota(iota_col[:, :], pattern=[[1, 1]], base=0,
                           channel_multiplier=1,
                           allow_small_or_imprecise_dtypes=True)
            nc.vector.tensor_scalar(ident[:, :], iota_row[:, :],
                                    iota_col[:, 0:1], None,
                                    op0=mybir.AluOpType.is_equal)
            nc.gpsimd.iota(iota512[:, :], pattern=[[1, K]], base=0,
                           channel_multiplier=0,
                           allow_small_or_imprecise_dtypes=True)
            nc.sync.dma_start(embT2B[:, :], et2_in[:, :])
            nc.vector.tensor_copy(embT2Br[:, :], embT2B[:, :])
            for xb in xt_bufs:
                nc.vector.memset(xb[64:65, :].bitcast(F32), 1.0)

            # ---------------- main loop ----------------
            x4 = x_in.rearrange("(tt p) d -> p tt d", p=128)
            enc4 = enc_o.rearrange("(tt p) k -> p tt k", p=128)
            q4 = q_o.rearrange("(tt p) d -> p tt d", p=128)
            x_b = enc_b = q_b = None
            for t in range(nt):
                j = t % 4
                if j == 0:
                    x_b = xp.tile([128, 4, D], F32, name="x_b")
                    nc.sync.dma_start(x_b[:, :, :], x4[:, t:t + 4, :])
                    enc_b = encp.tile([128, 4, K], F32, name="enc_b")
                    q_b = qp.tile([128, 4, D], F32, name="q_b")

                # xT via PE transpose; augmented lhsT row 64 = ones (pre-set)
                xt_ps = ptp.tile([64, 128], F32, tag="xt")
                nc.tensor.transpose(out=xt_ps[:, :], in_=x_b[:, j, :],
                                    identity=ident[:, :])
                xt = xt_bufs[t % 4]
                nc.scalar.activation(out=xt[0:64, :], in_=xt_ps[:, :],
                                     func=mybir.ActivationFunctionType.Copy)

                # s = x @ (2 emb)^T - B   [128, 512] PSUM (f32r matmul)
                m2 = pmp.tile([128, K], F32)
                nc.tensor.matmul(out=m2[:, :], lhsT=xt[:, :],
                                 rhs=embT2Br[:, :], start=True, stop=True)

                # s PSUM -> SBUF (ACT)
                s_sb = sp.tile([128, K], F32)
                nc.scalar.activation(out=s_sb[:, :], in_=m2[:, :],
                                     func=mybir.ActivationFunctionType.Copy)

                # argmax; max8 out doubles as the loss/gap output
                nc.vector.max(out=smax_all[:, 8 * t:8 * t + 8], in_=s_sb[:, :])
                nc.vector.max_index(out=idx_all[:, 8 * t:8 * t + 8],
                                    in_max=smax_all[:, 8 * t:8 * t + 8],
                                    in_values=s_sb[:, :])

                # one-hot: 2/3 of tiles on DVE (is_equal), 1/3 on ACT
                # (relu(1 - (iota - idx)^2), exact for integers)
                if t % 4 != 3:
                    idxf = smp.tile([128, 1], F32, tag="idxf")
                    nc.vector.tensor_copy(idxf[:, :],
                                          idx_all[:, 8 * t:8 * t + 1])
                    nc.vector.tensor_scalar(enc_b[:, j, :], iota512[:, :],
                                            idxf[:, 0:1], None,
                                            op0=mybir.AluOpType.is_equal)
                else:
                    idxnf = smp.tile([128, 1], F32, tag="idxnf")
                    nc.vector.tensor_scalar(idxnf[:, :],
                                            idx_all[:, 8 * t:8 * t + 1],
                                            -1.0, None,
                                            op0=mybir.AluOpType.mult)
                    d2 = sp.tile([128, K], F32, tag="d2")
                    nc.scalar.activation(
                        out=d2[:, :], in_=iota512[:, :],
                        func=mybir.ActivationFunctionType.Square,
                        bias=idxnf[:, 0:1], scale=1.0)
                    nc.scalar.activation(
                        out=enc_b[:, j, :], in_=d2[:, :],
                        func=mybir.ActivationFunctionType.Relu,
                        bias=1.0, scale=-1.0)

                # gather quantized rows: q[p] = emb[idx[p]]
                nc.gpsimd.indirect_dma_start(
                    out=q_b[:, j, :], out_offset=None, in_=e_in[:, :],
                    in_offset=bass.IndirectOffsetOnAxis(
                        ap=idx_all[:, 8 * t:8 * t + 1], axis=0))

                if j == 3:
                    t0 = t - 3
                    nc.sync.dma_start(enc4[:, t0:t0 + 4, :], enc_b[:, :, :])
                    nc.sync.dma_start(q4[:, t0:t0 + 4, :], q_b[:, :, :])

            # ---------------- tail ----------------
            nc.sync.dma_start(sm_o[:, :], smax_all[:, :])
            nc.sync.dma_start(idx_o[:, :], idx_all[:, :])

    if not nc.is_finalized():
        nc.finalize()
    return nc


_NC_CACHE = {}


def _get_nc(ns=NS):
    if ns not in _NC_CACHE:
        _NC_CACHE[ns] = build_nc(ns)
    return _NC_CACHE[ns]


def _host_prep(emb_w):
    embT2B = np.empty((D + 1, K), np.float32)
    embT2B[:D] = (np.float32(2.0) * emb_w).T
    bvec = np.sum(emb_w.astype(np.float32) ** 2, axis=1,
                  dtype=np.float32).astype(np.float32)
    embT2B[D] = -bvec
    return np.ascontiguousarray(embT2B), bvec


def kernel(inputs: np.ndarray, emb_w: np.ndarray, _trace=False):
    inputs = np.ascontiguousarray(np.asarray(inputs, dtype=np.float32))
    emb_w = np.ascontiguousarray(np.asarray(emb_w, dtype=np.float32))
    assert inputs.shape == (N, D) and emb_w.shape == (K, D)

    nc = _get_nc()
    embT2B, bvec = _host_prep(emb_w)
    in_maps = [
        {"x": inputs[i * NS:(i + 1) * NS], "emb": emb_w, "embT2B": embT2B}
        for i in range(N_CORES)
    ]
    out = run_bass_kernel_spmd(nc, in_maps, core_ids=list(range(N_CORES)),
                               trace=_trace)
    res = out.results

    enc = np.concatenate([r["enc"] for r in res], axis=0)
    q = np.concatenate([r["q"] for r in res], axis=0)
    nt = NS // 128
    # device row r of core i lives at [p=r%128, t=r//128]
    idx = np.concatenate([r["idx"][:, 0::8].T.reshape(-1) for r in res])
    smax0 = np.concatenate([r["sm"][:, 0::8].T.reshape(-1) for r in res])
    smax1 = np.concatenate([r["sm"][:, 1::8].T.reshape(-1) for r in res])

    # ---- host repair of near-tie rows: reproduce the reference's f32
    # rounding (distances at magnitude ~64 quantize to a ~7.6e-6 grid)
    flagged = np.flatnonzero((smax0 - smax1) < GAP_THRESHOLD)
    if flagged.size:
        xr = inputs[flagged]
        a32 = np.sum(xr ** 2, axis=1, dtype=np.float32).astype(np.float32)
        m32 = xr @ emb_w.T
        d32 = ((a32[:, None] + bvec[None, :]).astype(np.float32)
               - np.float32(2.0) * m32).astype(np.float32)
        idx_fix = np.argmin(d32, axis=1)
        changed = idx_fix != idx[flagged]
        rows = flagged[changed]
        if rows.size:
            idx[rows] = idx_fix[changed]
            enc[rows] = 0.0
            enc[rows, idx[rows]] = 1.0
            q[rows] = emb_w[idx[rows]]

    diff = q.astype(np.float64) - inputs.astype(np.float64)
    loss = np.float32(
        (1.0 + COMMITMENT_COST) * (np.einsum("ij,ij->", diff, diff) / (N * D)))

    counts = np.bincount(idx, minlength=K).astype(np.float64)
    avg_probs = counts / N
    perplexity = np.float32(
        np.exp(-np.sum(avg_probs * np.log(avg_probs + 1e-10))))

    # straight-through output, f32 exact as reference computes it
    quantized_st = inputs + (q - inputs)

    if _trace:
        return (loss, quantized_st, perplexity, enc), out
    return loss, quantized_st, perplexity, enc


# revision 39
# speedup vs baseline: 1.0301x; 1.0301x over previous
"""VQ codebook quantization kernel for 8 TRN2 NeuronCores.

Data-parallel: inputs [131072, 64] sharded 16384 rows/core, codebook [512, 64]
replicated. Per 128-row tile the device computes:
  - s = x @ (2*emb)^T - B   via one PE matmul with an augmented contraction
    row (lhsT row 64 = ones, rhs row 64 = -B), where B = sum(emb^2) per code
  - argmax(s) (max8 + max_index), one-hot encodings, quantized rows (indirect
    DMA gather), and per-row sum(x^2) for the loss.
The reference computes distances in f32 at magnitude ||x||^2 ~ 64, which
quantizes them to a ~7.6e-6 grid and creates near-ties; the device ranks by
the fine-grained s instead, and the host repairs the ~0.5% of rows whose
top-2 gap is below a threshold by recomputing them with the reference's exact
f32 rounding semantics.
"""

import numpy as np

import concourse.bass as bass
import concourse.mybir as mybir
from concourse.bacc import Bacc
from concourse.tile import TileContext
from concourse.bass_utils import run_bass_kernel_spmd

N_CORES = 8
N, D, K = 131072, 64, 512
NS = N // N_CORES            # rows per core
COMMITMENT_COST = 0.25
GAP_THRESHOLD = 5e-5         # flag rows for host repair

F32 = mybir.dt.float32
F32R = mybir.dt.float32r
U32 = mybir.dt.uint32


def build_nc(ns=NS):
    nt = ns // 128           # tiles per core
    nc = Bacc()
    x_in = nc.declare_dram_parameter("x", [ns, D], F32, isOutput=False)
    e_in = nc.declare_dram_parameter("emb", [K, D], F32, isOutput=False)
    et2_in = nc.declare_dram_parameter("embT2B", [D + 1, K], F32,
                                       isOutput=False)
    enc_o = nc.declare_dram_parameter("enc", [ns, K], F32, isOutput=True)
    q_o = nc.declare_dram_parameter("q", [ns, D], F32, isOutput=True)
    idx_o = nc.declare_dram_parameter("idx", [128, 8 * nt], U32, isOutput=True)
    sm_o = nc.declare_dram_parameter("sm", [128, 8 * nt], F32, isOutput=True)

    with TileContext(nc) as tc:
        with (
            tc.tile_pool(name="persist", bufs=1) as pp,
            tc.tile_pool(name="psum_t", bufs=3, space="PSUM") as ptp,
            tc.tile_pool(name="psum_m", bufs=3, space="PSUM") as pmp,
            tc.tile_pool(name="xp", bufs=4) as xp,
            tc.tile_pool(name="xtp", bufs=3) as xtp,
            tc.tile_pool(name="sp", bufs=4) as sp,
            tc.tile_pool(name="encp", bufs=3) as encp,
            tc.tile_pool(name="qp", bufs=3) as qp,
            tc.tile_pool(name="smallp", bufs=4) as smp,
        ):
            # ---------------- persistent tiles ----------------
            ident = pp.tile([128, 128], F32)        # identity for PE transpose
            embT2B = pp.tile([D + 1, K], F32)       # rows 0-63: (2emb)^T; 64: -B
            embT2Br = pp.tile([D + 1, K], F32R)     # f32r-rounded copy
            iota512 = pp.tile([128, K], F32)        # 0..511 each row
            smax_all = pp.tile([128, 8 * nt], F32)  # max8 outputs per tile
            idx_all = pp.tile([128, 8 * nt], U32)   # max_index outputs
            xt_bufs = [pp.tile([D + 1, 128], F32R, tag=f"xtb{i}",
                               name=f"xtb{i}")
                       for i in range(4)]

            # ---------------- setup ----------------
            iota_row = pp.tile([128, 128], F32)
            iota_col = pp.tile([128, 1], F32)
            nc.gpsimd.iota(iota_row[:, :], pattern=[[1, 128]], base=0,
                           channel_multiplier=0,
                           allow_small_or_imprecise_dtypes=True)
            nc.gpsimd.iota(iota_col[:, :], pattern=[[1, 1]], base=0,
                           channel_multiplier=1,
                           allow_small_or_imprecise_dtypes=True)
            nc.vector.tensor_scalar(ident[:, :], iota_row[:, :],
                                    iota_col[:, 0:1], None,
                                    op0=mybir.AluOpType.is_equal)
            nc.gpsimd.iota(iota512[:, :], pattern=[[1, K]], base=0,
                           channel_multiplier=0,
                           allow_small_or_imprecise_dtypes=True)
            nc.sync.dma_start(embT2B[:, :], et2_in[:, :])
            nc.vector.tensor_copy(embT2Br[:, :], embT2B[:, :])
            for xb in xt_bufs:
                nc.vector.memset(xb[64:65, :].bitcast(F32), 1.0)

            # ---------------- main loop ----------------
            x4 = x_in.rearrange("(tt p) d -> p tt d", p=128)
            enc4 = enc_o.rearrange("(tt p) k -> p tt k", p=128)
            q4 = q_o.rearrange("(tt p) d -> p tt d", p=128)
            x_b = enc_b = q_b = None
            for t in range(nt):
                j = t % 4
                if j == 0:
                    x_b = xp.tile([128, 4, D], F32, name="x_b")
                    nc.sync.dma_start(x_b[:, :, :], x4[:, t:t + 4, :])
                    enc_b = encp.tile([128, 4, K], F32, name="enc_b")
                    q_b = qp.tile([128, 4, D], F32, name="q_b")

                # xT via PE transpose; augmented lhsT row 64 = ones (pre-set)
                xt_ps = ptp.tile([64, 128], F32, tag="xt")
                nc.tensor.transpose(out=xt_ps[:, :], in_=x_b[:, j, :],
                                    identity=ident[:, :])
                xt = xt_bufs[t % 4]
                nc.scalar.activation(out=xt[0:64, :], in_=xt_ps[:, :],
                                     func=mybir.ActivationFunctionType.Copy)

                # s = x @ (2 emb)^T - B   [128, 512] PSUM (f32r matmul)
                m2 = pmp.tile([128, K], F32)
                nc.tensor.matmul(out=m2[:, :], lhsT=xt[:, :],
                                 rhs=embT2Br[:, :], start=True, stop=True)

                # s PSUM -> SBUF (ACT)
                s_sb = sp.tile([128, K], F32)
                nc.scalar.activation(out=s_sb[:, :], in_=m2[:, :],
                                     func=mybir.ActivationFunctionType.Copy)

                # argmax; max8 out doubles as the loss/gap output
                nc.vector.max(out=smax_all[:, 8 * t:8 * t + 8], in_=s_sb[:, :])
                nc.vector.max_index(out=idx_all[:, 8 * t:8 * t + 8],
                                    in_max=smax_all[:, 8 * t:8 * t + 8],
                                    in_values=s_sb[:, :])

                # one-hot: 2/3 of tiles on DVE (is_equal), 1/3 on ACT
                # (relu(1 - (iota - idx)^2), exact for integers)
                if t % 3 != 2:
                    idxf = smp.tile([128, 1], F32, tag="idxf")
                    nc.vector.tensor_copy(idxf[:, :],
                                          idx_all[:, 8 * t:8 * t + 1])
                    nc.vector.tensor_scalar(enc_b[:, j, :], iota512[:, :],
                                            idxf[:, 0:1], None,
                                            op0=mybir.AluOpType.is_equal)
                else:
                    idxnf = smp.tile([128, 1], F32, tag="idxnf")
                    nc.vector.tensor_scalar(idxnf[:, :],
                                            idx_all[:, 8 * t:8 * t + 1],
                                            -1.0, None,
                                            op0=mybir.AluOpType.mult)
                    d2 = sp.tile([128, K], F32, tag="d2")
                    nc.scalar.activation(
                        out=d2[:, :], in_=iota512[:, :],
                        func=mybir.ActivationFunctionType.Square,
                        bias=idxnf[:, 0:1], scale=1.0)
                    nc.scalar.activation(
                        out=enc_b[:, j, :], in_=d2[:, :],
                        func=mybir.ActivationFunctionType.Relu,
                        bias=1.0, scale=-1.0)

                # gather quantized rows: q[p] = emb[idx[p]]
                nc.gpsimd.indirect_dma_start(
                    out=q_b[:, j, :], out_offset=None, in_=e_in[:, :],
                    in_offset=bass.IndirectOffsetOnAxis(
                        ap=idx_all[:, 8 * t:8 * t + 1], axis=0))

                if j == 3:
                    t0 = t - 3
                    nc.sync.dma_start(enc4[:, t0:t0 + 4, :], enc_b[:, :, :])
                    nc.sync.dma_start(q4[:, t0:t0 + 4, :], q_b[:, :, :])

            # ---------------- tail ----------------
            nc.sync.dma_start(sm_o[:, :], smax_all[:, :])
            nc.sync.dma_start(idx_o[:, :], idx_all[:, :])

    if not nc.is_finalized():
        nc.finalize()
    return nc


_NC_CACHE = {}


def _get_nc(ns=NS):
    if ns not in _NC_CACHE:
        _NC_CACHE[ns] = build_nc(ns)
    return _NC_CACHE[ns]


def _host_prep(emb_w):
    embT2B = np.empty((D + 1, K), np.float32)
    embT2B[:D] = (np.float32(2.0) * emb_w).T
    bvec = np.sum(emb_w.astype(np.float32) ** 2, axis=1,
                  dtype=np.float32).astype(np.float32)
    embT2B[D] = -bvec
    return np.ascontiguousarray(embT2B), bvec


def kernel(inputs: np.ndarray, emb_w: np.ndarray, _trace=False):
    inputs = np.ascontiguousarray(np.asarray(inputs, dtype=np.float32))
    emb_w = np.ascontiguousarray(np.asarray(emb_w, dtype=np.float32))
    assert inputs.shape == (N, D) and emb_w.shape == (K, D)

    nc = _get_nc()
    embT2B, bvec = _host_prep(emb_w)
    in_maps = [
        {"x": inputs[i * NS:(i + 1) * NS], "emb": emb_w, "embT2B": embT2B}
        for i in range(N_CORES)
    ]
    out = run_bass_kernel_spmd(nc, in_maps, core_ids=list(range(N_CORES)),
                               trace=_trace)
    res = out.results

    enc = np.concatenate([r["enc"] for r in res], axis=0)
    q = np.concatenate([r["q"] for r in res], axis=0)
    nt = NS // 128
    # device row r of core i lives at [p=r%128, t=r//128]
    idx = np.concatenate([r["idx"][:, 0::8].T.reshape(-1) for r in res])
    smax0 = np.concatenate([r["sm"][:, 0::8].T.reshape(-1) for r in res])
    smax1 = np.concatenate([r["sm"][:, 1::8].T.reshape(-1) for r in res])

    # ---- host repair of near-tie rows: reproduce the reference's f32
    # rounding (distances at magnitude ~64 quantize to a ~7.6e-6 grid)
    flagged = np.flatnonzero((smax0 - smax1) < GAP_THRESHOLD)
    if flagged.size:
        xr = inputs[flagged]
        a32 = np.sum(xr ** 2, axis=1, dtype=np.float32).astype(np.float32)
        m32 = xr @ emb_w.T
        d32 = ((a32[:, None] + bvec[None, :]).astype(np.float32)
               - np.float32(2.0) * m32).astype(np.float32)
        idx_fix = np.argmin(d32, axis=1)
        changed = idx_fix != idx[flagged]
        rows = flagged[changed]
        if rows.size:
            idx[rows] = idx_fix[changed]
            enc[rows] = 0.0
            enc[rows, idx[rows]] = 1.0
            q[rows] = emb_w[idx[rows]]

    diff = q.astype(np.float64) - inputs.astype(np.float64)
    loss = np.float32(
        (1.0 + COMMITMENT_COST) * (np.einsum("ij,ij->", diff, diff) / (N * D)))

    counts = np.bincount(idx, minlength=K).astype(np.float64)
    avg_probs = counts / N
    perplexity = np.float32(
        np.exp(-np.sum(avg_probs * np.log(avg_probs + 1e-10))))

    # straight-through output, f32 exact as reference computes it
    quantized_st = inputs + (q - inputs)

    if _trace:
        return (loss, quantized_st, perplexity, enc), out
    return loss, quantized_st, perplexity, enc
